# revision 8
# baseline (speedup 1.0000x reference)
"""CRD loss kernel for Trainium2, 8-core data-parallel SPMD.

loss = -sum_i( (zs_i . zt_i) / (|zs_i| |zt_i|) ) / B
  zs = f_s @ W_s.T + b_s   [B, 128]
  zt = f_t @ W_t.T + b_t   [B, 128]

Sharding: batch B=16384 split across 8 cores (2048 rows each); projection
weights replicated. Each core computes its partial sum of normalized row
dots; host combines the 8 scalars.

Per-core dataflow (all fp32 data, fp32r matmul arithmetic):
  - x tiles [128, D] DMA'd naturally (rows on partitions).
  - PE transposes 128x128 blocks into PSUM; DVE/ACT copy them to SBUF as
    fp32r -> xT tiles [dim-chunk 128, 512 rows].
  - z.T [feat 128, rows 512] = sum_k (W.T chunk).T @ xT chunk, accumulated
    in PSUM; bias added via rank-1 matmul (b x ones_row).
  - zs.T/zt.T copied to SBUF (fp32r); DVE/ACT form zs*zt, zs^2, zt^2.
  - ones-vector matmuls column-sum those into [1, 512] rows of one PSUM
    tile (partitions 0/32/64): row dots, |zs|^2, |zt|^2.
  - tail: u = ss*tt (DVE), v = rsqrt(u) (ACT), then
    tensor_tensor_reduce(st * v) accumulates the block partial.
  - 4 block partials reduced to one scalar, DMA'd out.
"""
import numpy as np

import concourse.bass as bass
import concourse.mybir as mybir
from concourse.tile import TileContext
from concourse import bass_utils
from concourse.masks import make_identity

# Problem shapes (hardcoded per contest contract)
B = 16384
DS = 768
DT = 1024
F = 128
NCORES = 8
R = B // NCORES          # rows per core = 2048
BLK = 512                # row block (fp32 moving-operand max)
NBLK = R // BLK          # 4
P = 128

f32 = mybir.dt.float32
f32r = mybir.dt.float32r

_CACHE = {}


def legalize_waits(nc, max_waits=1):
    """Walrus codegen in this container rejects >1 sync-wait per instruction.
    Split extra waits onto same-engine NoOps placed right before the instr."""
    n_fixed = 0
    for fn in nc.m.functions:
        for blk in fn.blocks:
            new_insts = []
            for inst in blk.instructions:
                si = inst.sync_info
                if (
                    si is not None
                    and len(si.on_wait) > max_waits
                    and not isinstance(inst, mybir.InstISA)
                ):
                    waits = list(si.on_wait)
                    extra, keep = waits[:-max_waits], waits[-max_waits:]
                    for j, w in enumerate(extra):
                        nop = mybir.InstNoOp(
                            name=f"{inst.name}-wn{j}", engine=inst.engine
                        )
                        nop.sync_info = mybir.SyncInfo(on_wait=[w], on_update=[])
                        new_insts.append(nop)
                    inst.sync_info = mybir.SyncInfo(
                        on_wait=keep, on_update=list(si.on_update)
                    )
                    n_fixed += 1
                new_insts.append(inst)
            blk.instructions = new_insts
    return n_fixed


def build():
    nc = bass.Bass("TRN2")
    fs = nc.dram_tensor("fs", [R, DS], f32, kind="ExternalInput")
    ft = nc.dram_tensor("ft", [R, DT], f32, kind="ExternalInput")
    ws = nc.dram_tensor("ws", [F, DS], f32, kind="ExternalInput")
    wt = nc.dram_tensor("wt", [F, DT], f32, kind="ExternalInput")
    bs = nc.dram_tensor("bs", [1, F], f32, kind="ExternalInput")
    bt = nc.dram_tensor("bt", [1, F], f32, kind="ExternalInput")
    out = nc.dram_tensor("out", [1, 1], f32, kind="ExternalOutput")

    with TileContext(nc) as tc:
        with (
            tc.tile_pool(name="const", bufs=1) as const,
            tc.tile_pool(name="xnat_s", bufs=6) as xnat_s_pool,
            tc.tile_pool(name="xnat_t", bufs=6) as xnat_t_pool,
            tc.tile_pool(name="xT", bufs=6) as xT_pool,
            tc.tile_pool(name="zprod", bufs=4) as zprod_pool,
            tc.tile_pool(name="tail", bufs=2) as tail_pool,
            tc.tile_pool(name="psum_zs", bufs=1, space="PSUM") as psum_zs_pool,
            tc.tile_pool(name="psum_zt", bufs=2, space="PSUM") as psum_zt_pool,
            tc.tile_pool(name="psum_tp", bufs=2, space="PSUM") as psum_tp_pool,
            tc.tile_pool(name="psum_sum", bufs=1, space="PSUM") as psum_sum_pool,
        ):
            # ---- constants / weights prep ----
            identity = const.tile([P, P], f32)
            make_identity(nc, identity[:, :])

            ones_col_f = const.tile([P, 1], f32)
            nc.vector.memset(ones_col_f, 1.0)
            ones_col = const.tile([P, 1], f32r)
            nc.vector.tensor_copy(ones_col, ones_col_f)

            ones_row_f = const.tile([1, BLK], f32)
            nc.vector.memset(ones_row_f, 1.0)
            ones_row = const.tile([1, BLK], f32r)
            nc.vector.tensor_copy(ones_row, ones_row_f)

            ws_nat = const.tile([F, DS], f32)
            wt_nat = const.tile([F, DT], f32)
            nc.sync.dma_start(ws_nat, ws[:, :])
            nc.sync.dma_start(wt_nat, wt[:, :])

            bs_nat = const.tile([1, F], f32)
            bt_nat = const.tile([1, F], f32)
            nc.sync.dma_start(bs_nat, bs[:, :])
            nc.sync.dma_start(bt_nat, bt[:, :])
            bs_r = const.tile([1, F], f32r)
            bt_r = const.tile([1, F], f32r)
            nc.vector.tensor_copy(bs_r, bs_nat)
            nc.vector.tensor_copy(bt_r, bt_nat)

            # W.T chunks, fp32r: wT[:, k*128:(k+1)*128] = W[:, chunk k].T
            wsT = const.tile([P, DS], f32r)
            wtT = const.tile([P, DT], f32r)
            for w_nat, w_T, D in ((ws_nat, wsT, DS), (wt_nat, wtT, DT)):
                nch = D // P
                for k0 in range(0, nch, 4):
                    kw = min(4, nch - k0)
                    tp = psum_tp_pool.tile([P, BLK], f32, tag="tp")
                    for j in range(kw):
                        k = k0 + j
                        nc.tensor.transpose(
                            tp[:, j * P:(j + 1) * P],
                            w_nat[:, k * P:(k + 1) * P],
                            identity,
                        )
                    nc.vector.tensor_copy(
                        w_T[:, k0 * P:(k0 + kw) * P], tp[:, : kw * P]
                    )

            partials = const.tile([1, NBLK], f32)

            # ---- main loop over row blocks ----
            for blk in range(NBLK):
                psum_z = {}
                for br, (x_dram, D, w_T, b_r, xpool) in {
                    "s": (fs, DS, wsT, bs_r, xnat_s_pool),
                    "t": (ft, DT, wtT, bt_r, xnat_t_pool),
                }.items():
                    nch = D // P
                    x_tiles = []
                    for rt in range(4):
                        xn = xpool.tile([P, D], f32, tag=f"xn_{br}")
                        r0 = blk * BLK + rt * P
                        nc.sync.dma_start(xn, x_dram[r0:r0 + P, :])
                        x_tiles.append(xn)

                    psz = (psum_zs_pool if br == "s" else psum_zt_pool).tile(
                        [P, BLK], f32
                    )
                    psum_z[br] = psz
                    for k in range(nch):
                        tp = psum_tp_pool.tile([P, BLK], f32, tag="tp")
                        for rt in range(4):
                            nc.tensor.transpose(
                                tp[:, rt * P:(rt + 1) * P],
                                x_tiles[rt][:, k * P:(k + 1) * P],
                                identity,
                            )
                        xT = xT_pool.tile([P, BLK], f32r, tag="xT")
                        if k % 2 == 0:
                            nc.vector.tensor_copy(xT, tp)
                        else:
                            nc.scalar.copy(xT, tp)
                        nc.tensor.matmul(
                            psz,
                            w_T[:, k * P:(k + 1) * P],
                            xT,
                            start=(k == 0),
                            stop=False,
                        )
                    # bias: rank-1 update b (x) ones_row
                    nc.tensor.matmul(psz, b_r, ones_row, start=False, stop=True)

                # products and squares (fp32r SBUF)
                zs_sb = zprod_pool.tile([P, BLK], f32r, tag="zsb")
                zt_sb = zprod_pool.tile([P, BLK], f32r, tag="zsb")
                nc.vector.tensor_copy(zs_sb, psum_z["s"])
                nc.scalar.copy(zt_sb, psum_z["t"])

                prod_st = zprod_pool.tile([P, BLK], f32r, tag="prod")
                zs2 = zprod_pool.tile([P, BLK], f32r, tag="prod")
                zt2 = zprod_pool.tile([P, BLK], f32r, tag="prod")
                nc.vector.tensor_mul(prod_st, zs_sb, zt_sb)
                nc.scalar.square(zs2, zs_sb)
                nc.scalar.square(zt2, zt_sb)

                # column sums via ones-matmuls into bank-aligned free slices
                # of one PSUM tile (all at partition 0): [st | ss | tt]
                sums = psum_sum_pool.tile([1, 3 * BLK], f32)
                nc.tensor.matmul(
                    sums[0:1, 0:BLK], ones_col, prod_st, start=True, stop=True
                )
                nc.tensor.matmul(
                    sums[0:1, BLK:2 * BLK], ones_col, zs2, start=True, stop=True
                )
                nc.tensor.matmul(
                    sums[0:1, 2 * BLK:3 * BLK], ones_col, zt2, start=True, stop=True
                )

                # tail: partial_blk = sum_r st[r] / sqrt(ss[r]*tt[r])
                sums_sb = tail_pool.tile([1, 3 * BLK], f32, tag="sums_sb")
                nc.scalar.copy(sums_sb, sums)
                u = tail_pool.tile([1, BLK], f32, tag="u")
                nc.vector.tensor_mul(
                    u, sums_sb[0:1, BLK:2 * BLK], sums_sb[0:1, 2 * BLK:3 * BLK]
                )
                uinv = tail_pool.tile([1, BLK], f32, tag="uinv")
                nc.vector.reciprocal(uinv, u)
                v = tail_pool.tile([1, BLK], f32, tag="v")
                nc.scalar.activation(
                    v, uinv, mybir.ActivationFunctionType.Sqrt
                )
                w_ = tail_pool.tile([1, BLK], f32, tag="w")
                nc.vector.tensor_mul(w_, sums_sb[0:1, 0:BLK], v)
                nc.vector.reduce_sum(
                    partials[0:1, blk:blk + 1], w_, axis=mybir.AxisListType.X
                )

            total = const.tile([1, 1], f32)
            nc.vector.reduce_sum(total, partials, axis=mybir.AxisListType.X)
            nc.sync.dma_start(out[:, :], total)

    legalize_waits(nc)
    return nc


def get_nc():
    if "nc" not in _CACHE:
        _CACHE["nc"] = build()
    return _CACHE["nc"]


def make_in_maps(f_s, f_t, W_s, b_s, W_t, b_t):
    f_s = np.ascontiguousarray(np.asarray(f_s, dtype=np.float32))
    f_t = np.ascontiguousarray(np.asarray(f_t, dtype=np.float32))
    W_s = np.ascontiguousarray(np.asarray(W_s, dtype=np.float32))
    b_s = np.ascontiguousarray(np.asarray(b_s, dtype=np.float32)).reshape(1, F)
    W_t = np.ascontiguousarray(np.asarray(W_t, dtype=np.float32))
    b_t = np.ascontiguousarray(np.asarray(b_t, dtype=np.float32)).reshape(1, F)
    in_maps = []
    for c in range(NCORES):
        sl = slice(c * R, (c + 1) * R)
        in_maps.append(
            {
                "fs": f_s[sl],
                "ft": f_t[sl],
                "ws": W_s,
                "wt": W_t,
                "bs": b_s,
                "bt": b_t,
            }
        )
    return in_maps


def combine(results):
    partials = np.array(
        [results[c]["out"][0, 0] for c in range(NCORES)], dtype=np.float64
    )
    loss = -(partials.sum() / B)
    return np.array([loss], dtype=np.float32)


def kernel(f_s, f_t, W_s, b_s, W_t, b_t):
    nc = get_nc()
    in_maps = make_in_maps(f_s, f_t, W_s, b_s, W_t, b_t)
    res = bass_utils.run_bass_kernel_spmd(
        nc, in_maps, core_ids=list(range(NCORES))
    )
    return combine(res.results)
